# revision 7
# baseline (speedup 1.0000x reference)
"""Trainium2 Bass kernel for nn_AttentionPool1d (sliding-window self-attention pool).

Math (faithful to the reference):
    xp = pad(x, 4 each side on T)                    # [B, D, S], S = T + 8
    Y[:, s]  = Wq @ xp[:, s] + bq                    # Q and K share this projection
    Z[:, s]  = Wo @ xp[:, s]                         # V is raw xp; Wo commutes with the
                                                     #   attention average (linearity)
    energy[t, w] = Y[:, t+4] . Y[:, t+w] / (sqrt(D) * 1.5)
    attn = softmax_w(energy)
    out[:, t] = sum_w attn[t, w] * Z[:, t+w] + bo    # since sum_w attn = 1

Distribution: pure data-parallel over batch, 2 batches per NeuronCore, 8 cores.

Precision plan (validated on HW vs the fp32 reference, seed-0 inputs):
    - Q/K path in fp8 (DoubleRow matmuls); host pre-scales x by 8 and Wq by 64
      to stay out of the fp8 subnormal range; rescales fold into PSUM evacs.
    - V path in fp16 (fp8 V fails the 2e-2 budget).  ZT carries 8Z; the
      1/8 + bo fold into the F evacuation.

Key structure (v3):
    - The diagonal 9-bands of the 6 grams in a band pair are extracted
      ON-CHIP by a reverse gpsimd local_scatter (per-partition indices,
      negative entries skipped): e_all[t, 9k+w] = g_all[t, 128k+t+w].  No
      DRAM round trip, ~2us total latency for the softmax chain.
    - The banded matrix Ab^T is built by a second local_scatter; its PE
      transposes are deferred to aggregation time so the in-order PE queue
      never parks on the scatter's latency chain.
    - Both batches' emission streams are interleaved (b1 offset B1_OFS
      steps) and each batch interleaves Act-heavy Y evacuations with
      PE-heavy Z blocks, so no in-order engine queue starves another.
    - PSUM-evacuation work is split between Act and DVE by a cost-weighted
      balancer; transpose copies merge 3-up; F staging is per-(group,
      ec-pair) so each output DMA waits only its own evacuations.
    - PSUM banks: proj 4 + f 2 + gram/trp shared ring 2 = 8.
    - x8/xp load as per-chunk tiles so DMA-completion semaphores stay
      fine-grained; descriptor generation is spread across the serialized
      HWDGE and the Pool SWDGE queue; an early dummy activation pre-fires
      the 1.3us activation-table load under the x-load phase.
"""

import math
from contextlib import ExitStack

import numpy as np
import ml_dtypes

import concourse.bass as bass
import concourse.tile as tile
from concourse import bacc, mybir
from concourse.bass_utils import run_bass_kernel_spmd
from concourse.masks import make_identity

f32 = mybir.dt.float32
fp16 = mybir.dt.float16
fp8 = mybir.dt.float8e4
i16 = mybir.dt.int16
DR = mybir.MatmulPerfMode.DoubleRow

B, D, T = 16, 512, 2048
NCORES = 8
BPC = B // NCORES  # batches per core
PAD = 4
W = 9
S = T + 2 * PAD  # 2056
SCALE = 1.0 / (math.sqrt(D) * 1.5)
YPAD = 4  # extra left shift of the yt layout (fp8 DR ldweights offset headroom)
SY_LEN = S + 2 * YPAD

SX = 8.0  # host pre-scale on x (fp16/fp8 carry 8x)
SW = 64.0  # host pre-scale on Wq (fp8 carries 64Wq)
SY = 8.0  # scale carried by Y8 (fp8 carries 8Y)

P = 128
DC = D // P  # 4 chunks of the hidden dim
TB = 120  # t-block size; window = TB + 8 = 128 fits one contraction
NBLK = (T + TB - 1) // TB  # 18
GB = 3  # t-blocks per agg group (PSUM / output staging granularity)
NGRP = NBLK // GB  # 6
PB = 2 * GB  # t-blocks per band pair (DRAM/scatter batching)
NPAIR = NBLK // PB  # 3

REPS = 1  # device-side repeat count (timing amplification only)
UNROLL = False  # python-unroll REPS instead of tc.For_i (sim analysis only)
USE_SWDGE = True  # issue bulk loads via the Pool SWDGE queue (else HWDGE)

# gram packing: the 6 grams of a band pair land side by side in one
# [128, 768] tile; a reverse local_scatter extracts the diagonal 9-bands.
G_WPITCH = PB * P  # 768

NIDX = PB * W + 2  # 56 scatter indices per partition (padded even, 2 dummies)

# 5 Y-projection s-tiles; pair pi's gram reads need Y columns up to
# min(TB*(PB*pi + PB-1) + PAD + 128, S) = 732 / 1452 / 2056, satisfied after
# tiles 1 / 3 / 4.
_S_TILES = [(0, 344), (344, 448), (792, 448), (1240, 448), (1688, 368)]


def _body(nc, tc, ctx, x, x8in, wq, bq, wo, bo, sidx, gidx, y):
    singles = ctx.enter_context(tc.tile_pool(name="singles", bufs=1))

    # pre-fire the activation-table load under the x-load phase
    warm = singles.tile([1, 2], f32)
    nc.vector.memset(warm, 0.0)
    nc.scalar.activation(
        out=warm, in_=warm, func=mybir.ActivationFunctionType.Identity, scale=1.0
    )

    ident_b = singles.tile([P, P], fp16)

    # weights arrive pre-transposed (and pre-scaled) from the host:
    # wq is (64*Wq).T in fp8, wo is Wo.T in fp16
    wqT = [
        singles.tile([P, 2, D], fp8, name=f"wqTh{h}") for h in range(2)
    ]  # [d_part, d_chunk-pair, e]
    woT = singles.tile([P, DC, D], fp16)
    bq_sb = singles.tile([P, DC], f32)  # carries 8*bq
    bo_sb = singles.tile([P, DC], f32)
    sidx_sb = singles.tile([P, NIDX], i16)  # per-partition scatter indices
    gidx_sb = singles.tile([P, G_WPITCH], i16)  # diag-band extraction indices

    # ---------------- pools ----------------
    xp_pool = ctx.enter_context(tc.tile_pool(name="xp", bufs=2))
    x8_pool = ctx.enter_context(tc.tile_pool(name="x8", bufs=2))
    y_pool = ctx.enter_context(tc.tile_pool(name="ypool", bufs=2))
    zt_pool = ctx.enter_context(tc.tile_pool(name="zt", bufs=2))
    small = ctx.enter_context(tc.tile_pool(name="small", bufs=NPAIR + 2))
    abp = ctx.enter_context(tc.tile_pool(name="abp", bufs=NGRP + 2))
    fsb_pool = ctx.enter_context(tc.tile_pool(name="fsb", bufs=3))
    proj_ps = ctx.enter_context(tc.tile_pool(name="proj_ps", bufs=4, space="PSUM"))
    # gram tiles ([128,4,128] f32, 1 bank) and merged transpose tiles
    # ([128,3,128] fp16) share one 2-buf ring (same tag): per pair the ring
    # is g(h0), g(h1), tr(gA), tr(gB) with copies draining between reuses.
    gt_ps = ctx.enter_context(tc.tile_pool(name="gt_ps", bufs=2, space="PSUM"))
    f_ps = ctx.enter_context(tc.tile_pool(name="f_ps", bufs=2, space="PSUM"))

    st = {}  # per-batch state
    dge = nc.gpsimd if USE_SWDGE else nc.sync
    eng_ns = {"act": 0.0, "dve": 0.0}

    def pick_engine(nelem):
        # cost-weighted balance of PSUM-evac work between Act and DVE
        act_c = 0.833 * nelem + 143
        dve_c = 1.0417 * nelem + 125
        if eng_ns["act"] + act_c <= eng_ns["dve"] + dve_c:
            eng_ns["act"] += act_c
            return "act"
        eng_ns["dve"] += dve_c
        return "dve"

    def load_weights_early():
        wv = wq.rearrange("(c p) e -> p c e", p=P)
        for h in range(2):
            dge.dma_start(wqT[h], wv[:, 2 * h : 2 * h + 2, :])

    def load_weights_mid():
        nc.sync.dma_start(woT, wo.rearrange("(c p) e -> p c e", p=P))

    def load_weights_late():
        dge.dma_start(bq_sb, bq.rearrange("(c p) -> p c", p=P))

    def load_bo():
        dge.dma_start(bo_sb, bo.rearrange("(c p) -> p c", p=P))

    def load_x8(bi, split_first=False, eng=None, mid=None):
        # one tile per d-chunk so DMA-completion semaphores stay fine-grained
        # (a single multi-chunk tile makes consumers wait for ALL chunk DMAs)
        eng = eng or nc.sync
        x8 = [
            x8_pool.tile([P, 2, S], fp8, tag=f"x8h{h}", name=f"x8h{h}")
            for h in range(2)
        ]
        s = st.setdefault(bi, {})
        s["x8"] = x8
        for h in range(2):
            nc.vector.memset(x8[h][:, :, 0:PAD], 0.0)
            nc.vector.memset(x8[h][:, :, S - PAD : S], 0.0)
        xv8 = x8in[bi].rearrange("(c p) t -> p c t", p=P)
        spans = ((0, 800), (800, T)) if split_first else ((0, T),)
        for t0, t1 in spans:
            for h in range(2):
                eng.dma_start(
                    out=x8[h][:, :, PAD + t0 : PAD + t1],
                    in_=xv8[:, 2 * h : 2 * h + 2, t0:t1],
                )
            if mid is not None and t0 == 0:
                mid(bi)

    def load_xp(bi, eng=None):
        # SWDGE (Pool) descriptor-gen runs parallel to the serialized HWDGE
        eng = eng or dge
        xp = [
            xp_pool.tile([P, S], fp16, tag=f"xp{dc}", name=f"xp{dc}")
            for dc in range(DC)
        ]
        s = st.setdefault(bi, {})
        s["xp"] = xp
        xv = x[bi].rearrange("(c p) t -> p c t", p=P)
        for dc in range(DC):
            nc.vector.memset(xp[dc][:, 0:PAD], 0.0)
            nc.vector.memset(xp[dc][:, S - PAD : S], 0.0)
        for dc in range(DC):
            eng.dma_start(out=xp[dc][:, PAD : PAD + T], in_=xv[:, dc, :])

    def init_batch(bi):
        s = st[bi]
        s["yt"] = y_pool.tile([P, DC, SY_LEN], fp8, tag="yt", name="yt")
        s["zt"] = zt_pool.tile([P, NBLK, D], fp16, tag="zt", name="zt")
        s["pend"] = {}

    def y_tile(bi, ti):
        # Y8^T = fp8(8*(Wq @ x + bq)) [e_part, e_chunk, s] for one s-tile
        s = st[bi]
        x8, yt = s["x8"], s["yt"]
        s0, sn = _S_TILES[ti]
        for ec in range(DC):
            ps = proj_ps.tile([P, 512], f32, tag="proj")
            for i in range(2):
                nc.tensor.matmul(
                    ps[:, 0:sn],
                    wqT[i][:, :, ec * P : (ec + 1) * P],
                    x8[i][:, :, s0 : s0 + sn],
                    start=(i == 0),
                    stop=(i == 1),
                    perf_mode=DR,
                )
            # Y8 = PSUM/64 + 8bq  (PSUM = 512*(Y - bq))
            out_ap = yt[:, ec, YPAD + s0 : YPAD + s0 + sn]
            if pick_engine(sn) == "act":
                nc.scalar.activation(
                    out=out_ap,
                    in_=ps[:, 0:sn],
                    func=mybir.ActivationFunctionType.Identity,
                    bias=bq_sb[:, ec : ec + 1],
                    scale=1.0 / SW,
                )
            else:
                nc.vector.tensor_scalar(
                    out=out_ap,
                    in0=ps[:, 0:sn],
                    scalar1=1.0 / SW,
                    scalar2=bq_sb[:, ec : ec + 1],
                    op0=mybir.AluOpType.mult,
                    op1=mybir.AluOpType.add,
                )

    def z_block(bi, ib):
        # ZT[s, e] for one 128-wide block at stride TB (fp16, carries 8Z)
        s = st[bi]
        xp, zt = s["xp"], s["zt"]
        s0z = TB * ib
        snz = min(P, S - s0z)
        ps = proj_ps.tile([P, 512], f32, tag="proj")
        for dc in range(DC):
            nc.tensor.matmul(
                ps[0:snz, :],
                xp[dc][:, s0z : s0z + snz],
                woT[:, dc, :],
                start=(dc == 0),
                stop=(dc == DC - 1),
            )
        if snz < P:
            nc.vector.memset(zt[:, ib, :], 0.0)
        if pick_engine(512) == "dve":
            nc.vector.tensor_copy(out=zt[0:snz, ib, :], in_=ps[0:snz, :])
        else:
            nc.scalar.copy(out=zt[0:snz, ib, :], in_=ps[0:snz, :])

    def band_pair(bi, pi):
        # 6 banded fp8 grams -> one DRAM pitch-trick round trip for the diag
        # bands -> softmax -> banded matrix via gpsimd local_scatter (on-chip)
        # -> PE transposes (3-up merged into one PSUM tile per agg group)
        s = st[bi]
        yt, pend = s["yt"], s["pend"]
        blocks = []
        for k in range(PB):
            t0 = TB * (pi * PB + k)
            tw = min(TB, T - t0)
            blocks.append((t0, tw))
        full = all(tw == TB for _, tw in blocks)

        g_all = small.tile([P, G_WPITCH], fp16, tag="gall")
        if not full:
            nc.vector.memset(g_all, 0.0)
        # 4 gram blocks share one 2KB PSUM bank -> one copy per tile.
        for h in range(2):
            g_ps = gt_ps.tile([P, 4, P], f32, tag="gram")
            nsl = 4 if h == 0 else 2
            for sl in range(nsl):
                k = h * 4 + sl
                t0, tw = blocks[k]
                sw = tw + 2 * PAD
                mw = min(P, S - (t0 + PAD))
                for i in range(2):
                    nc.tensor.matmul(
                        g_ps[0:mw, sl, 0:sw],
                        yt[:, 2 * i : 2 * i + 2, YPAD + t0 + PAD : YPAD + t0 + PAD + mw],
                        yt[:, 2 * i : 2 * i + 2, YPAD + t0 : YPAD + t0 + sw],
                        start=(i == 0),
                        stop=(i == 1),
                        perf_mode=DR,
                    )
            if full:
                dst = g_all[:, h * 4 * P : (h * 4 + nsl) * P]
                if pick_engine(nsl * P) == "act":
                    nc.scalar.copy(out=dst, in_=g_ps[:, 0:nsl, :])
                else:
                    nc.vector.tensor_copy(out=dst, in_=g_ps[:, 0:nsl, :])
            else:
                for sl in range(nsl):
                    k = h * 4 + sl
                    t0, tw = blocks[k]
                    sw = tw + 2 * PAD
                    mw = min(P, S - (t0 + PAD))
                    nc.vector.tensor_copy(
                        out=g_all[0:mw, k * P : k * P + sw],
                        in_=g_ps[0:mw, sl, 0:sw],
                    )

        # on-chip diag-band extraction: e_all[t, 9k+w] = g_all[t, 128k+t+w]
        # via a reverse local_scatter (per-partition indices; -1 entries of
        # gidx are ignored, so exactly the 54 band values land per partition)
        e_all = small.tile([P, PB, W], fp16, tag="eall")
        nc.gpsimd.local_scatter(
            e_all, g_all, gidx_sb, channels=P, num_elems=PB * W, num_idxs=G_WPITCH
        )
        s.setdefault("eall", {})[pi] = e_all

    def band_pair_b(bi, pi):
        # softmax + scatter, emitted a few schedule steps after band_pair so
        # the in-order Act/DVE queues don't park on the DMA round trip
        s = st[bi]
        pend = s["pend"]
        e_all = s["eall"].pop(pi)
        # softmax over the 9-wide window (values are small; no max-sub).
        # gram PSUM carries 64*E, folded into the Exp scale.
        eexp = small.tile([TB, PB, W], f32, tag="eexp")
        nc.scalar.activation(
            out=eexp,
            in_=e_all[0:TB],
            func=mybir.ActivationFunctionType.Exp,
            scale=SCALE / (SY * SY),
        )
        ssum = small.tile([TB, PB], f32, tag="ssum")
        nc.vector.reduce_sum(out=ssum, in_=eexp, axis=mybir.AxisListType.X)
        nc.vector.reciprocal(out=ssum, in_=ssum)
        attn = small.tile([P, NIDX], fp16, tag="attn")
        # rows 120..127 and the two pad columns are never scattered (idx -1)
        # but must be finite; zero the whole tile first
        nc.vector.memset(attn, 0.0)
        for k in range(PB):
            nc.vector.tensor_scalar_mul(
                attn[0:TB, k * W : (k + 1) * W],
                eexp[:, k, :],
                ssum[:, k : k + 1],
            )

        # banded matrix Ab^T[t, 128k + s] = attn[t, k, s - t] built on-chip.
        # The PE transposes are deferred to agg time: emitting them here would
        # park the in-order PE queue on the scatter's long latency chain.
        abts = small.tile([P, PB * P], fp16, tag="abts")
        nc.gpsimd.local_scatter(
            abts, attn, sidx_sb, channels=P, num_elems=PB * P, num_idxs=NIDX
        )
        pend[pi] = abts

    def agg_group(bi, gi):
        # F[e-chunk, t] = ZT_chunk^T @ Aband (= 8*out); evacuate as
        # out = F/8 + bo into the two-group staging tile, flushed by one
        # output DMA per pair of groups.
        s = st[bi]
        zt = s["zt"]
        abts = s["pend"][gi // 2]
        g2 = gi % 2
        trp = gt_ps.tile([P, GB, P], fp16, tag="gram", name="trp")
        for g in range(GB):
            k = g2 * GB + g
            nc.tensor.transpose(
                trp[:, g, 0:TB],
                abts[0:TB, k * P : (k + 1) * P],
                ident_b[0:TB, 0:TB],
            )
        ab = abp.tile([P, GB, TB], fp16, tag="ab")
        eng_ns["dve"] += 0.52 * GB * TB + 125
        nc.vector.tensor_copy(out=ab, in_=trp[:, :, 0:TB])
        abs_ = [(ab, g, TB * (gi * GB + g), min(TB, T - TB * (gi * GB + g))) for g in range(GB)]
        # two staging tiles (ec pairs) per group so each flush DMA only
        # waits the evacuations of its own half (tile-granular semaphores)
        f_all = [
            fsb_pool.tile([P, 2, GB * TB], fp16, tag=f"fall{h}", name=f"fall{h}")
            for h in range(2)
        ]
        hoff = 0
        for ec in range(DC):
            f_psum = f_ps.tile([P, GB * TB], f32, tag="fps")
            for ab, g, t0, tw in abs_:
                jb = gi * GB + g
                nc.tensor.matmul(
                    f_psum[:, g * TB : g * TB + tw],
                    zt[:, jb, ec * P : (ec + 1) * P],
                    ab[:, g, 0:tw],
                    start=True,
                    stop=True,
                )
            # full-width evac; cols past T-coverage hold stale-but-finite
            # PSUM data and are never flushed to DRAM
            dst = f_all[ec // 2][:, ec % 2, hoff : hoff + GB * TB]
            if pick_engine(GB * TB) == "act":
                nc.scalar.activation(
                    out=dst,
                    in_=f_psum,
                    func=mybir.ActivationFunctionType.Identity,
                    bias=bo_sb[:, ec : ec + 1],
                    scale=1.0 / SX,
                )
            else:
                nc.vector.tensor_scalar(
                    out=dst,
                    in0=f_psum,
                    scalar1=1.0 / SX,
                    scalar2=bo_sb[:, ec : ec + 1],
                    op0=mybir.AluOpType.mult,
                    op1=mybir.AluOpType.add,
                )
        tg0 = TB * GB * gi
        ext = min(TB * GB * (gi + 1), T) - tg0
        yv = y[bi].rearrange("(c p) t -> p c t", p=P)
        nc.sync.dma_start(yv[:, 0:2, tg0 : tg0 + ext], f_all[0][:, :, 0:ext])
        nc.sync.dma_start(yv[:, 2:4, tg0 : tg0 + ext], f_all[1][:, :, 0:ext])

    def batch_steps(bi, skip_first=False):
        # one batch's emission stream as atomic steps; ordering keeps pair
        # DMA/softmax chains and aggs well after their producers so no
        # in-order engine queue parks on a latency chain
        steps = []
        add = steps.append
        if not skip_first:
            add(lambda: init_batch(bi))
            add(lambda: y_tile(bi, 0))
            add(lambda: y_tile(bi, 1))
        add(lambda: band_pair(bi, 0))
        for z in (0, 1, 2):
            add(lambda z=z: z_block(bi, z))
        add(lambda: y_tile(bi, 2))
        add(lambda: band_pair_b(bi, 0))
        for z in (3, 4, 5):
            add(lambda z=z: z_block(bi, z))
        add(lambda: y_tile(bi, 3))
        add(lambda: agg_group(bi, 0))
        add(lambda: band_pair(bi, 1))
        for z in (6, 7, 8):
            add(lambda z=z: z_block(bi, z))
        add(lambda: agg_group(bi, 1))
        add(lambda: band_pair_b(bi, 1))
        add(lambda: y_tile(bi, 4))
        add(lambda: band_pair(bi, 2))
        for z in (9, 10, 11):
            add(lambda z=z: z_block(bi, z))
        add(lambda: agg_group(bi, 2))
        add(lambda: band_pair_b(bi, 2))
        for z in (12, 13, 14):
            add(lambda z=z: z_block(bi, z))
        add(lambda: agg_group(bi, 3))
        for z in (15, 16):
            add(lambda z=z: z_block(bi, z))
        add(lambda: agg_group(bi, 4))
        add(lambda: z_block(bi, 17))
        add(lambda: agg_group(bi, 5))
        return steps

    def first_chunk_work(bi):
        # emitted between the x8 first- and second-chunk DMAs: woT load plus
        # the first two Y s-tiles (they read only s<792, within chunk 1, and
        # emitting them here keeps their DMA-wait semaphores off chunk 2)
        load_weights_mid()
        load_weights_late()
        init_batch(bi)
        y_tile(bi, 0)
        y_tile(bi, 1)

    B1_OFS = 24  # b1 steps start after this many b0 steps

    def pipeline(_i=None):
        load_weights_early()
        load_x8(0, split_first=True, mid=first_chunk_work)
        dge.dma_start(gidx_sb, gidx)
        load_xp(0)
        dge.dma_start(sidx_sb, sidx)
        make_identity(nc, ident_b)
        s0 = batch_steps(0, skip_first=True)
        s1 = batch_steps(1)
        # hooks keyed by b0 step index (s0 is missing the 3 init steps)
        hooks = {
            5: load_bo,                            # after band_pair_b(0, 0)
            9: lambda: load_x8(1, eng=dge),  # ~after y_tile(0, 3)
            16: lambda: load_xp(1),                # mid z-phase of b0
        }
        out_i = 0
        for i, step in enumerate(s0):
            step()
            if i in hooks:
                hooks[i]()
            if i >= B1_OFS and out_i < len(s1):
                s1[out_i]()
                out_i += 1
        while out_i < len(s1):
            s1[out_i]()
            out_i += 1

    if REPS == 1:
        pipeline()
    elif UNROLL:
        for _ in range(REPS):
            pipeline()
    else:
        with tc.For_i(0, REPS, 1):
            pipeline()


def build_nc():
    nc = bacc.Bacc("TRN2", debug=False)
    x_in = nc.dram_tensor("x", [BPC, D, T], fp16, kind="ExternalInput")
    x8_in = nc.dram_tensor("x8", [BPC, D, T], fp8, kind="ExternalInput")
    wq_in = nc.dram_tensor("WqT", [D, D], fp8, kind="ExternalInput")
    bq_in = nc.dram_tensor("bq", [D], f32, kind="ExternalInput")
    wo_in = nc.dram_tensor("WoT", [D, D], fp16, kind="ExternalInput")
    bo_in = nc.dram_tensor("bo", [D], f32, kind="ExternalInput")
    sidx_in = nc.dram_tensor("sidx", [P, NIDX], i16, kind="ExternalInput")
    gidx_in = nc.dram_tensor("gidx", [P, G_WPITCH], i16, kind="ExternalInput")
    y_out = nc.dram_tensor("y", [BPC, D, T], fp16, kind="ExternalOutput")

    with tile.TileContext(nc) as tc, ExitStack() as ctx:
        _body(
            nc,
            tc,
            ctx,
            x_in.ap(),
            x8_in.ap(),
            wq_in.ap(),
            bq_in.ap(),
            wo_in.ap(),
            bo_in.ap(),
            sidx_in.ap(),
            gidx_in.ap(),
            y_out.ap(),
        )
    nc.compile()
    return nc


_NC_CACHE = []


def _get_nc():
    if not _NC_CACHE:
        _NC_CACHE.append(build_nc())
    return _NC_CACHE[0]


def _scatter_idx():
    idx = np.full((P, NIDX), -1, np.int16)
    tau = np.arange(TB)[:, None]
    for k in range(PB):
        for w_ in range(W):
            idx[0:TB, k * W + w_ : k * W + w_ + 1] = P * k + tau + w_
    return idx


def _gather_idx():
    # gidx[p, 128k + p + w] = 9k + w  (else -1): reverse-scatter extraction
    idx = np.full((P, G_WPITCH), -1, np.int16)
    for p in range(P):
        for k in range(PB):
            for w_ in range(W):
                j = P * k + p + w_
                if j < G_WPITCH:
                    idx[p, j] = W * k + w_
    return idx


def _in_maps(x, Wq, bq, Wo, bo):
    xs = np.asarray(x, dtype=np.float32) * SX
    x16 = np.ascontiguousarray(xs.astype(np.float16))
    x8 = np.ascontiguousarray(xs.astype(ml_dtypes.float8_e4m3))
    WqT = np.ascontiguousarray(
        (np.asarray(Wq, dtype=np.float32).T * SW).astype(ml_dtypes.float8_e4m3)
    )
    bq = np.ascontiguousarray(np.asarray(bq, dtype=np.float32) * SY)
    WoT = np.ascontiguousarray(np.asarray(Wo, dtype=np.float32).T.astype(np.float16))
    bo = np.ascontiguousarray(np.asarray(bo, dtype=np.float32))
    sidx = _scatter_idx()
    gidx = _gather_idx()
    return [
        {
            "x": x16[c * BPC : (c + 1) * BPC],
            "x8": x8[c * BPC : (c + 1) * BPC],
            "WqT": WqT,
            "bq": bq,
            "WoT": WoT,
            "bo": bo,
            "sidx": sidx,
            "gidx": gidx,
        }
        for c in range(NCORES)
    ]


def run(trace=False, **inputs):
    nc = _get_nc()
    res = run_bass_kernel_spmd(
        nc, _in_maps(**inputs), core_ids=list(range(NCORES)), trace=trace
    )
    out = np.concatenate([r["y"] for r in res.results], axis=0).astype(np.float32)
    return out, res


def kernel(x, Wq, bq, Wo, bo):
    out, _ = run(x=x, Wq=Wq, bq=bq, Wo=Wo, bo=bo)
    return out
